# revision 25
# baseline (speedup 1.0000x reference)
"""GAT (2-layer, 4-head) Bass kernel for Trainium2, data-parallel over 8 NeuronCores.

Math (per sample b, per attention instance with weights W, a = [a1; a2]):
    Wh = h @ W                      [N, F]
    s  = Wh @ a1   (per-dst-node i score part)
    t  = Wh @ a2   (per-src-node j score part)
    e[i,j]   = leaky_relu(s[i] + t[j], 0.2)
    att      = softmax_j(where(adj[i,j] > 0, e, -9e15))
    out[i]   = sum_j att[i,j] * Wh[j]

Key factorization: exp(lrelu(z)) = max(e^z, e^{0.2 z}) for z = s_i + t_j, so
    p[j,i] = m * max(e^{s_i} e^{t_j}, e^{0.2 s_i} e^{0.2 t_j})
           = m * e^{0.2 s_i} * max(e^{0.8 s_i + t_j}, e^{0.2 t_j})
The e^{0.2 s_i} factor is constant along the softmax axis (j) and cancels in
normalization, so the kernel computes only
    p'[j,i] = m[j,i] * max(G[i], r[j]) * F[j]
with G = e^{0.8 s} (one [128,N] ACT exp per instance, via the PE-replicated
s matmul), r = e^{-0.8 t}, F = e^{t} (tiny per-node columns). Per N^2-tile:
one DVE tensor_scalar (max with r-column, mult by F-column) and one native
tensor_tensor mask multiply (2x DVE perf mode). No custom DVE ops, no
per-tile ACT work.

Attention-apply orientation: the contraction over j runs with the p' tile
[j, i-chunk] as the PE stationary and the small [Wh | ones] block moving, so
the output lands as O[i, blk, f] with the softmax row-sum in column 64 --
i.e. BOTH the output and the row-sum are per-i-PARTITION. The reciprocal
then runs on a [128, 8] column (free-size 8, ~100x cheaper than a [1, N]
row) and normalization fuses into the PSUM->SBUF copy as ACT Copy with a
per-partition scale AP. Layer-1 heads are transposed back to h_cat^T
[feat, i] with PE transpose blocks; layer 2 consumes O[i, f] directly
(elu elementwise, mean over nodes via a PE ones-column contraction).

Scheduling: each instance is split into phase1 (s matmul + G exp + score
tensor_scalars + mask multiplies -> p'), phase2 (the 64 attention matmuls),
and tail (reciprocal + normalize + transpose). The emission order software-
pipelines phase1 two instances ahead of phase2 and interleaves the next
sample's L1 with the current sample's L2, so the in-order engine queues
stay full across the layer joins. PSUM is budgeted to exactly 8 banks:
s-halves [128,512] (1 bank x2 bufs), transpose staging (1 x2), attention
outputs split at the bank boundary into two [128,4,65] tiles (1 x2 x2).
"""

import os
import sys

import numpy as np

if not os.path.isdir(os.path.join(os.path.dirname(os.path.abspath(__file__)), "concourse")):
    for _p in ("/opt/trn_rl_repo", os.path.expanduser("~/.axon_site/_ro/trn_rl_repo")):
        if os.path.isdir(_p) and _p not in sys.path:
            sys.path.append(_p)

import ml_dtypes  # noqa: E402

import concourse.bacc as bacc  # noqa: E402
import concourse.tile as tile  # noqa: E402
from concourse import mybir  # noqa: E402
from concourse.bass_utils import run_bass_kernel_spmd  # noqa: E402

BF16 = ml_dtypes.bfloat16

B, N, FIN, FH, H, FOUT = 16, 1024, 256, 64, 4, 64
NCORES = 8
SPC = B // NCORES  # samples per core
KT = FIN // 128    # k tiles (2)
JT = N // 128      # j tiles (8)
IB = N // 128      # i chunks (8)
HB = IB // 2       # i chunks per PSUM tile
ALPHA = 0.2

F32 = mybir.dt.float32
F16 = mybir.dt.float16
BF = mybir.dt.bfloat16
AF = mybir.ActivationFunctionType
OP = mybir.AluOpType
AX = mybir.AxisListType


class _Inst:
    """One attention instance (a head of L1, or L2), emitted in 3 phases."""

    def __init__(self, nc, pools, maskT_sb, spec, out_dt, emit_out):
        self.nc, self.pools, self.maskT_sb = nc, pools, maskT_sb
        self.spec, self.out_dt, self.emit_out = spec, out_dt, emit_out

    def phase1a(self):
        """s matmul halves + G exp halves (PE + ACT front-matter)."""
        nc, spec = self.nc, self.spec
        work, psA = self.pools["work"], self.pools["psA"]

        self.g16 = work.tile([128, N], BF, tag="g16", name="g16")
        for ih in range(2):
            sb_ps = psA.tile([128, 512], F32, tag="big", name="sbh")
            for kt in range(KT):
                nc.tensor.matmul(
                    sb_ps,
                    spec["rep"](kt),
                    spec["rhs"](kt)[:, ih * 512 : (ih + 1) * 512],
                    start=(kt == 0),
                    stop=(kt == KT - 1),
                )
            nc.scalar.activation(
                self.g16[:, ih * 512 : (ih + 1) * 512], sb_ps, AF.Exp, scale=0.8
            )

    def phase1b(self):
        """Score tensor_scalars + mask multiplies -> p' tile (DVE)."""
        nc, spec = self.nc, self.spec
        workbig = self.pools["workbig"]
        pT = workbig.tile([128, JT, N], BF, tag="pt", name="pT")
        self.pT = pT
        g16 = self.g16
        for jt in range(JT):
            nc.vector.tensor_scalar(
                pT[:, jt, :], g16, spec["rcol"](jt), spec["fcol"](jt), OP.max, OP.mult
            )
        for ih in range(2):
            half = slice(ih * (JT // 2), (ih + 1) * (JT // 2))
            nc.vector.tensor_tensor(
                pT[:, half, :], pT[:, half, :], self.maskT_sb[:, half, :], OP.mult
            )

    def phase2(self):
        """O[i, blk, f] (+ rowsum col 64): p' chunks stationary, Wh moving."""
        nc, spec = self.nc, self.spec
        psO = self.pools["psO"]
        self.ot_ps = [
            psO.tile([128, HB, FH + 1], F32, tag=f"ot{half}", name=f"ot{half}")
            for half in range(2)
        ]
        for ib in range(IB):
            for jt in range(JT):
                nc.tensor.matmul(
                    self.ot_ps[ib // HB][:, ib % HB, :],
                    self.pT[:, jt, ib * 128 : (ib + 1) * 128],
                    spec["wh"](jt),
                    start=(jt == 0),
                    stop=(jt == JT - 1),
                )

    def tail(self):
        """Per-partition reciprocal of rowsum cols; normalization rides the
        PSUM->SBUF copies as an ACT per-partition scale."""
        nc = self.nc
        work = self.pools["work"]
        rsc = work.tile([128, IB], F32, tag="rsc", name="rsc")
        for half in range(2):
            nc.vector.tensor_copy(
                rsc[:, half * HB : (half + 1) * HB], self.ot_ps[half][:, :, FH]
            )
        rbc = work.tile([128, IB], F32, tag="rbc", name="rbc")
        nc.vector.reciprocal_approx_fast(out=rbc, in_=rsc)
        o_norm = work.tile([128, IB, FH], self.out_dt, tag="onrm", name="onrm")
        for ib in range(IB):
            nc.scalar.activation(
                o_norm[:, ib, :], self.ot_ps[ib // HB][:, ib % HB, 0:FH], AF.Copy,
                scale=rbc[:, ib : ib + 1],
            )
        self.emit_out(o_norm)


def _build_nc():
    nc = bacc.Bacc()

    xT_d = nc.declare_dram_parameter("xT", [SPC, KT, 128, N], BF, isOutput=False)
    maskT_d = nc.declare_dram_parameter("maskT", [SPC, JT, 128, N], BF, isOutput=False)
    wbig1_d = nc.declare_dram_parameter("wbig1", [KT, 128, H * 65 + H], BF, isOutput=False)
    warep1_d = nc.declare_dram_parameter("warep1", [KT, 128, H * 128], BF, isOutput=False)
    wbig2_d = nc.declare_dram_parameter("wbig2", [KT, 128, 66], BF, isOutput=False)
    warep2_d = nc.declare_dram_parameter("warep2", [KT, 128, 128], BF, isOutput=False)
    ident_d = nc.declare_dram_parameter("ident", [128, 128], BF, isOutput=False)
    out_d = nc.declare_dram_parameter("out", [SPC, FOUT], F32, isOutput=True)

    with tile.TileContext(nc) as tc:
        with (
            tc.tile_pool(name="const", bufs=1) as constp,
            tc.tile_pool(name="samp", bufs=2) as samp,
            tc.tile_pool(name="workbig", bufs=4) as workbig,
            tc.tile_pool(name="work", bufs=4) as work,
            tc.tile_pool(name="tail", bufs=1) as tailp,
            tc.tile_pool(name="psA", bufs=2, space="PSUM") as psA,
            tc.tile_pool(name="psT", bufs=2, space="PSUM") as psT,
            tc.tile_pool(name="psO", bufs=2, space="PSUM") as psO,
        ):
            pools = {"work": work, "workbig": workbig, "psA": psA, "psO": psO}

            wbig1_sb = constp.tile([128, KT, H * 65 + H], BF)
            warep1_sb = constp.tile([128, KT, H * 128], BF)
            wbig2_sb = constp.tile([128, KT, 66], BF)
            warep2_sb = constp.tile([128, KT, 128], BF)
            ident_sb = constp.tile([128, 128], BF)
            for kt in range(KT):
                nc.sync.dma_start(out=warep1_sb[:, kt, :], in_=warep1_d[kt])
                nc.sync.dma_start(out=wbig1_sb[:, kt, :], in_=wbig1_d[kt])
            ones128_sb = constp.tile([128, 1], BF)
            nc.vector.memset(ones128_sb, 1.0)

            # Per-sample state built lazily by the unit functions below.
            st = [dict() for _ in range(SPC)]

            def WH1(s):
                """DMA inputs; L1 Wh pass for all heads; r/F columns."""
                d = st[s]
                xT_sb = samp.tile([128, KT, N], BF, tag="xt", name="xt")
                for kt in range(KT):
                    nc.sync.dma_start(out=xT_sb[:, kt, :], in_=xT_d[s, kt])
                maskT_sb = samp.tile([128, JT, N], BF, tag="mask", name="mask")
                for jt in range(JT):
                    nc.sync.dma_start(out=maskT_sb[:, jt, :], in_=maskT_d[s, jt])
                if s == 0:
                    for kt in range(KT):
                        nc.sync.dma_start(out=wbig2_sb[:, kt, :], in_=wbig2_d[kt])
                        nc.sync.dma_start(out=warep2_sb[:, kt, :], in_=warep2_d[kt])
                    nc.sync.dma_start(out=ident_sb, in_=ident_d[:, :])
                whsb1 = samp.tile([128, JT, H * 65], BF, tag="whsb1", name="whsb1")
                tc1 = samp.tile([128, JT, H], F32, tag="tc1", name="tc1")
                for jt in range(JT):
                    wm_ps = psA.tile([128, H * 65 + H], F32, tag="big", name="wm")
                    for kt in range(KT):
                        nc.tensor.matmul(
                            wm_ps,
                            xT_sb[:, kt, jt * 128 : (jt + 1) * 128],
                            wbig1_sb[:, kt, :],
                            start=(kt == 0),
                            stop=(kt == KT - 1),
                        )
                    nc.scalar.copy(whsb1[:, jt, :], wm_ps[:, 0 : H * 65])
                    nc.vector.memset(whsb1[:, jt, FH : H * 65 : 65], 1.0)
                    nc.scalar.copy(tc1[:, jt, :], wm_ps[:, H * 65 : H * 65 + H])
                rc1 = samp.tile([128, JT, H], F32, tag="rc1", name="rc1")
                fc1 = samp.tile([128, JT, H], F32, tag="fc1", name="fc1")
                nc.scalar.activation(rc1, tc1, AF.Exp, scale=-0.8)
                nc.scalar.activation(fc1, tc1, AF.Exp, scale=1.0)
                hcatT = samp.tile([128, KT, N], BF, tag="hcat", name="hcat")
                d.update(xT_sb=xT_sb, maskT_sb=maskT_sb, whsb1=whsb1,
                         rc1=rc1, fc1=fc1, hcatT=hcatT)
                d["insts"] = {}
                for h in range(H):
                    def emit_l1(o_norm, d=d, h=h):
                        tp_ps = psT.tile([FH, IB, 128], BF, tag="tp", name="tp")
                        for ib in range(IB):
                            nc.tensor.transpose(
                                tp_ps[:, ib, :], o_norm[:, ib, :], ident_sb
                            )
                        dst = d["hcatT"][(h % 2) * 64 : (h % 2) * 64 + 64, h // 2, :]
                        nc.scalar.copy(dst, tp_ps)

                    d["insts"][h] = _Inst(
                        nc, pools, maskT_sb,
                        {
                            "rep": lambda kt, h=h: warep1_sb[:, kt, h * 128 : (h + 1) * 128],
                            "rhs": lambda kt, d=d: d["xT_sb"][:, kt, :],
                            "wh": lambda jt, d=d, h=h: d["whsb1"][:, jt, h * 65 : (h + 1) * 65],
                            "rcol": lambda jt, d=d, h=h: d["rc1"][:, jt, h : h + 1],
                            "fcol": lambda jt, d=d, h=h: d["fc1"][:, jt, h : h + 1],
                        },
                        BF, emit_l1,
                    )
                o2h = {}
                d["o2h"] = o2h

                def emit_l2(o_norm, o2h=o2h):
                    o2h["o2n"] = o_norm

                d["insts"]["L2"] = _Inst(
                    nc, pools, maskT_sb,
                    {
                        "rep": lambda kt: warep2_sb[:, kt, :],
                        "rhs": lambda kt, d=d: d["hcatT"][:, kt, :],
                        "wh": lambda jt, d=d: d["whsb2"][:, jt, :],
                        "rcol": lambda jt, d=d: d["rc2"][:, jt, :],
                        "fcol": lambda jt, d=d: d["fc2"][:, jt, :],
                    },
                    F32, emit_l2,
                )

            def WH2a(s):
                """L2 Wh matmuls; t columns via DVE; r/F exps. The whsb2
                SBUF copies are deferred to WH2b (they gate only P2(L2)),
                keeping the TL(s,3) -> P1b(s,L2) chain short."""
                d = st[s]
                whsb2 = samp.tile([128, JT, 65], BF, tag="whsb2", name="whsb2")
                tc2 = samp.tile([128, JT, 1], F32, tag="tc2", name="tc2")
                hcatT = d["hcatT"]
                d["wm2"] = []
                for jt in range(JT):
                    wm_ps = psA.tile([128, 66], F32, tag="big", name="wm2")
                    d["wm2"].append(wm_ps)
                    for kt in range(KT):
                        nc.tensor.matmul(
                            wm_ps,
                            hcatT[:, kt, jt * 128 : (jt + 1) * 128],
                            wbig2_sb[:, kt, :],
                            start=(kt == 0),
                            stop=(kt == KT - 1),
                        )
                    nc.vector.tensor_copy(tc2[:, jt, :], wm_ps[:, 65:66])
                rc2 = samp.tile([128, JT, 1], F32, tag="rc2", name="rc2")
                fc2 = samp.tile([128, JT, 1], F32, tag="fc2", name="fc2")
                nc.scalar.activation(rc2, tc2, AF.Exp, scale=-0.8)
                nc.scalar.activation(fc2, tc2, AF.Exp, scale=1.0)
                d.update(whsb2=whsb2, rc2=rc2, fc2=fc2)

            def WH2b(s):
                d = st[s]
                whsb2 = d["whsb2"]
                for jt in range(JT):
                    nc.scalar.copy(whsb2[:, jt, 0:FOUT], d["wm2"][jt][:, 0:FOUT])
                    nc.vector.memset(whsb2[:, jt, FOUT : FOUT + 1], 1.0)
                d["wm2"] = None

            def ELU(s):
                """elu(x) = relu(x) + min(exp(x), 1) - 1; -1 folded into the
                post-reduce scale. Mean over nodes (= partitions) on the PE."""
                d = st[s]
                o2n = d["o2h"]["o2n"]
                ex = tailp.tile([128, IB, FH], F32, tag="ex", name="ex")
                nc.scalar.activation(ex, o2n, AF.Exp)
                bmax = tailp.tile([128, IB, FH], F32, tag="bmax", name="bmax")
                nc.scalar.activation(bmax, o2n, AF.Relu)
                eluv = tailp.tile([128, IB, FH], BF, tag="eluv", name="eluv")
                nc.vector.scalar_tensor_tensor(eluv, ex, 1.0, bmax, OP.min, OP.add)
                mean_ps = psA.tile([FH, 1], F32, tag="big", name="mean")
                for ib in range(IB):
                    nc.tensor.matmul(
                        mean_ps,
                        eluv[:, ib, :],
                        ones128_sb,
                        start=(ib == 0),
                        stop=(ib == IB - 1),
                    )
                outc = tailp.tile([FH, 1], F32, tag="outc", name="outc")
                nc.vector.tensor_scalar(outc, mean_ps, 1.0 / N, -1.0, OP.mult, OP.add)
                nc.sync.dma_start(out=out_d[s].rearrange("(f a) -> f a", a=1), in_=outc)

            def P1a(s, k):
                st[s]["insts"][k].phase1a()

            def P1b(s, k):
                st[s]["insts"][k].phase1b()

            def P2(s, k):
                st[s]["insts"][k].phase2()

            def TL(s, k):
                st[s]["insts"][k].tail()

            # ---- emission schedule: sample 0 launches before sample 1's
            # Wh pass (the warmup is DMA-bound); thereafter the two samples
            # interleave at instance granularity. P1a (s-matmul + G) runs
            # ahead of P1b so every G precedes the tails' ACT copies in the
            # in-order queues. At the L2 join, P1a(L2) (which needs only
            # h_cat) precedes the L2 Wh pass, and the whsb2 SBUF copies
            # (WH2b) are deferred off the join-critical path.
            WH1(0)
            P1a(0, 0); P1a(0, 1)
            P1b(0, 0)
            WH1(1)
            P1a(1, 0); P1a(1, 1)
            P1b(1, 0)
            P2(0, 0); P1a(0, 2); P1b(0, 1)
            P2(1, 0); P1a(1, 2); P1b(1, 1)
            TL(0, 0); P2(0, 1); P1a(0, 3); P1b(0, 2)
            TL(1, 0); P2(1, 1); P1a(1, 3); P1b(1, 2)
            TL(0, 1); P2(0, 2); P1b(0, 3)
            TL(1, 1); P2(1, 2); P1b(1, 3)
            TL(0, 2); P2(0, 3)
            TL(1, 2); P2(1, 3)
            TL(0, 3); P1a(0, "L2"); WH2a(0)
            TL(1, 3); P1a(1, "L2"); WH2a(1)
            P1b(0, "L2"); WH2b(0)
            P1b(1, "L2"); WH2b(1)
            P2(0, "L2")
            P2(1, "L2")
            TL(0, "L2"); ELU(0)
            TL(1, "L2"); ELU(1)

    nc.finalize()
    return nc


_NC_CACHE = None


def _prep_host(x, adj, W_heads, a_heads, W_out, a_out):
    xT = np.ascontiguousarray(np.asarray(x, np.float32).transpose(0, 2, 1)).astype(BF16)
    xT = xT.reshape(B, KT, 128, N)
    maskT = (np.asarray(adj) > 0).transpose(0, 2, 1).astype(BF16)  # [B, j, i]
    maskT = np.ascontiguousarray(maskT).reshape(B, JT, 128, N)

    W_heads = np.asarray(W_heads, np.float32)
    a_heads = np.asarray(a_heads, np.float32)
    W_out = np.asarray(W_out, np.float32)
    a_out = np.asarray(a_out, np.float32)

    wbig1 = np.zeros((FIN, H * 65 + H), dtype=np.float32)
    warep1 = np.zeros((FIN, H * 128), dtype=np.float32)
    for h in range(H):
        Wh_ = W_heads[h]
        wbig1[:, h * 65 : h * 65 + FH] = Wh_
        wbig1[:, H * 65 + h] = Wh_ @ a_heads[h, FH:, 0]
        warep1[:, h * 128 : (h + 1) * 128] = (Wh_ @ a_heads[h, :FH, 0])[:, None]
    wbig2 = np.zeros((FIN, 66), dtype=np.float32)
    wbig2[:, 0:FOUT] = W_out
    wbig2[:, 65] = W_out @ a_out[FOUT:, 0]
    warep2 = np.repeat((W_out @ a_out[:FOUT, 0])[:, None], 128, axis=1)

    shared = {
        "wbig1": wbig1.astype(BF16).reshape(KT, 128, H * 65 + H),
        "warep1": warep1.astype(BF16).reshape(KT, 128, H * 128),
        "wbig2": wbig2.astype(BF16).reshape(KT, 128, 66),
        "warep2": warep2.astype(BF16).reshape(KT, 128, 128),
        "ident": np.eye(128, dtype=np.float32).astype(BF16),
    }
    in_maps = []
    for c in range(NCORES):
        sl = slice(c * SPC, (c + 1) * SPC)
        m = {"xT": np.ascontiguousarray(xT[sl]), "maskT": np.ascontiguousarray(maskT[sl])}
        m.update(shared)
        in_maps.append(m)
    return in_maps


def kernel(x, adj, W_heads, a_heads, W_out, a_out, _trace=False):
    global _NC_CACHE
    if _NC_CACHE is None:
        _NC_CACHE = _build_nc()
    nc = _NC_CACHE
    in_maps = _prep_host(x, adj, W_heads, a_heads, W_out, a_out)
    res = run_bass_kernel_spmd(nc, in_maps, core_ids=list(range(NCORES)), trace=_trace)
    out = np.concatenate([res.results[c]["out"] for c in range(NCORES)], axis=0)
    if _trace:
        kernel._last_results = res
    return out.astype(np.float32)


# revision 26
# speedup vs baseline: 1.1716x; 1.1716x over previous
"""GAT (2-layer, 4-head) Bass kernel for Trainium2, data-parallel over 8 NeuronCores.

Math (per sample b, per attention instance with weights W, a = [a1; a2]):
    Wh = h @ W                      [N, F]
    s  = Wh @ a1   (per-dst-node i score part)
    t  = Wh @ a2   (per-src-node j score part)
    e[i,j]   = leaky_relu(s[i] + t[j], 0.2)
    att      = softmax_j(where(adj[i,j] > 0, e, -9e15))
    out[i]   = sum_j att[i,j] * Wh[j]

Key factorization: exp(lrelu(z)) = max(e^z, e^{0.2 z}) for z = s_i + t_j, so
    p[j,i] = m * max(e^{s_i} e^{t_j}, e^{0.2 s_i} e^{0.2 t_j})
           = m * e^{0.2 s_i} * max(e^{0.8 s_i + t_j}, e^{0.2 t_j})
The e^{0.2 s_i} factor is constant along the softmax axis (j) and cancels in
normalization, so the kernel computes only
    p'[j,i] = m[j,i] * max(G[i], r[j]) * F[j]
with G = e^{0.8 s} (one [128,N] ACT exp per instance, via the PE-replicated
s matmul), r = e^{-0.8 t}, F = e^{t} (tiny per-node columns). Per N^2-tile:
one DVE tensor_scalar (max with r-column, mult by F-column) and one native
tensor_tensor mask multiply (2x DVE perf mode). No custom DVE ops, no
per-tile ACT work.

Attention-apply orientation: the contraction over j runs with the p' tile
[j, i-chunk] as the PE stationary and the small [Wh | ones] block moving, so
the output lands as O[i, blk, f] with the softmax row-sum in column 64 --
i.e. BOTH the output and the row-sum are per-i-PARTITION. The reciprocal
then runs on a [128, 8] column (free-size 8, ~100x cheaper than a [1, N]
row) and normalization fuses into the PSUM->SBUF copy as ACT Copy with a
per-partition scale AP. Layer-1 heads are transposed back to h_cat^T
[feat, i] with PE transpose blocks; layer 2 consumes O[i, f] directly
(elu elementwise, mean over nodes via a PE ones-column contraction).

Scheduling: each instance is split into phase1 (s matmul + G exp + score
tensor_scalars + mask multiplies -> p'), phase2 (the 64 attention matmuls),
and tail (reciprocal + normalize + transpose). The emission order software-
pipelines phase1 two instances ahead of phase2 and interleaves the next
sample's L1 with the current sample's L2, so the in-order engine queues
stay full across the layer joins. PSUM is budgeted to exactly 8 banks:
s-halves [128,512] (1 bank x2 bufs), transpose staging (1 x2), attention
outputs split at the bank boundary into two [128,4,65] tiles (1 x2 x2).
"""

import os
import sys

import numpy as np

if not os.path.isdir(os.path.join(os.path.dirname(os.path.abspath(__file__)), "concourse")):
    for _p in ("/opt/trn_rl_repo", os.path.expanduser("~/.axon_site/_ro/trn_rl_repo")):
        if os.path.isdir(_p) and _p not in sys.path:
            sys.path.append(_p)

import ml_dtypes  # noqa: E402

import concourse.bacc as bacc  # noqa: E402
import concourse.tile as tile  # noqa: E402
from concourse import mybir  # noqa: E402
from concourse.bass_utils import run_bass_kernel_spmd  # noqa: E402

BF16 = ml_dtypes.bfloat16

B, N, FIN, FH, H, FOUT = 16, 1024, 256, 64, 4, 64
NCORES = 8
SPC = B // NCORES  # samples per core
KT = FIN // 128    # k tiles (2)
JT = N // 128      # j tiles (8)
IB = N // 128      # i chunks (8)
HB = IB // 2       # i chunks per PSUM tile
ALPHA = 0.2

F32 = mybir.dt.float32
F16 = mybir.dt.float16
BF = mybir.dt.bfloat16
AF = mybir.ActivationFunctionType
OP = mybir.AluOpType
AX = mybir.AxisListType


class _Inst:
    """One attention instance (a head of L1, or L2), emitted in 3 phases."""

    def __init__(self, nc, pools, maskT_sb, spec, out_dt, emit_out):
        self.nc, self.pools, self.maskT_sb = nc, pools, maskT_sb
        self.spec, self.out_dt, self.emit_out = spec, out_dt, emit_out

    def phase1a(self):
        """s matmul halves + G exp halves (PE + ACT front-matter)."""
        nc, spec = self.nc, self.spec
        work, psA = self.pools["work"], self.pools["psA"]

        self.g16 = work.tile([128, N], BF, tag="g16", name="g16")
        for ih in range(2):
            sb_ps = psA.tile([128, 512], F32, tag="big", name="sbh")
            for kt in range(KT):
                nc.tensor.matmul(
                    sb_ps,
                    spec["rep"](kt),
                    spec["rhs"](kt)[:, ih * 512 : (ih + 1) * 512],
                    start=(kt == 0),
                    stop=(kt == KT - 1),
                )
            nc.scalar.activation(
                self.g16[:, ih * 512 : (ih + 1) * 512], sb_ps, AF.Exp, scale=0.8
            )

    def phase1b(self):
        """Score tensor_scalars + mask multiplies -> p' tile (DVE)."""
        nc, spec = self.nc, self.spec
        workbig = self.pools["workbig"]
        pT = workbig.tile([128, JT, N], BF, tag="pt", name="pT")
        self.pT = pT
        g16 = self.g16
        for jt in range(JT):
            nc.vector.tensor_scalar(
                pT[:, jt, :], g16, spec["rcol"](jt), spec["fcol"](jt), OP.max, OP.mult
            )
        for ih in range(2):
            half = slice(ih * (JT // 2), (ih + 1) * (JT // 2))
            nc.vector.tensor_tensor(
                pT[:, half, :], pT[:, half, :], self.maskT_sb[:, half, :], OP.mult
            )

    def phase2(self):
        """O[i, blk, f] (+ rowsum col 64): p' chunks stationary, Wh moving."""
        nc, spec = self.nc, self.spec
        psO = self.pools["psO"]
        self.ot_ps = [
            psO.tile([128, HB, FH + 1], F32, tag=f"ot{half}", name=f"ot{half}")
            for half in range(2)
        ]
        for ib in range(IB):
            for jt in range(JT):
                nc.tensor.matmul(
                    self.ot_ps[ib // HB][:, ib % HB, :],
                    self.pT[:, jt, ib * 128 : (ib + 1) * 128],
                    spec["wh"](jt),
                    start=(jt == 0),
                    stop=(jt == JT - 1),
                )

    def tail(self):
        """Per-partition reciprocal of rowsum cols; normalization rides the
        PSUM->SBUF copies as an ACT per-partition scale."""
        nc = self.nc
        work = self.pools["work"]
        rsc = work.tile([128, IB], F32, tag="rsc", name="rsc")
        for half in range(2):
            nc.vector.tensor_copy(
                rsc[:, half * HB : (half + 1) * HB], self.ot_ps[half][:, :, FH]
            )
        rbc = work.tile([128, IB], F32, tag="rbc", name="rbc")
        nc.vector.reciprocal_approx_fast(out=rbc, in_=rsc)
        o_norm = work.tile([128, IB, FH], self.out_dt, tag="onrm", name="onrm")
        for ib in range(IB):
            nc.scalar.activation(
                o_norm[:, ib, :], self.ot_ps[ib // HB][:, ib % HB, 0:FH], AF.Copy,
                scale=rbc[:, ib : ib + 1],
            )
        self.emit_out(o_norm)


def _build_nc():
    nc = bacc.Bacc()

    xT_d = nc.declare_dram_parameter("xT", [SPC, KT, 128, N], BF, isOutput=False)
    maskT_d = nc.declare_dram_parameter("maskT", [SPC, JT, 128, N], BF, isOutput=False)
    wbig1_d = nc.declare_dram_parameter("wbig1", [KT, 128, H * 65 + H], BF, isOutput=False)
    warep1_d = nc.declare_dram_parameter("warep1", [KT, 128, H * 128], BF, isOutput=False)
    wbig2_d = nc.declare_dram_parameter("wbig2", [KT, 128, 66], BF, isOutput=False)
    warep2_d = nc.declare_dram_parameter("warep2", [KT, 128, 128], BF, isOutput=False)
    ident_d = nc.declare_dram_parameter("ident", [128, 128], BF, isOutput=False)
    out_d = nc.declare_dram_parameter("out", [SPC, FOUT], F32, isOutput=True)

    with tile.TileContext(nc) as tc:
        with (
            tc.tile_pool(name="const", bufs=1) as constp,
            tc.tile_pool(name="samp", bufs=2) as samp,
            tc.tile_pool(name="workbig", bufs=4) as workbig,
            tc.tile_pool(name="work", bufs=4) as work,
            tc.tile_pool(name="tail", bufs=1) as tailp,
            tc.tile_pool(name="psA", bufs=2, space="PSUM") as psA,
            tc.tile_pool(name="psT", bufs=2, space="PSUM") as psT,
            tc.tile_pool(name="psO", bufs=2, space="PSUM") as psO,
        ):
            pools = {"work": work, "workbig": workbig, "psA": psA, "psO": psO}

            wbig1_sb = constp.tile([128, KT, H * 65 + H], BF)
            warep1_sb = constp.tile([128, KT, H * 128], BF)
            wbig2_sb = constp.tile([128, KT, 66], BF)
            warep2_sb = constp.tile([128, KT, 128], BF)
            ident_sb = constp.tile([128, 128], BF)
            for kt in range(KT):
                nc.sync.dma_start(out=warep1_sb[:, kt, :], in_=warep1_d[kt])
                nc.sync.dma_start(out=wbig1_sb[:, kt, :], in_=wbig1_d[kt])
            ones128_sb = constp.tile([128, 1], BF)
            nc.vector.memset(ones128_sb, 1.0)

            # Per-sample state built lazily by the unit functions below.
            st = [dict() for _ in range(SPC)]

            def WH1(s):
                """DMA inputs; L1 Wh pass for all heads; r/F columns."""
                d = st[s]
                xT_sb = samp.tile([128, KT, N], BF, tag="xt", name="xt")
                for kt in range(KT):
                    nc.sync.dma_start(out=xT_sb[:, kt, :], in_=xT_d[s, kt])
                maskT_sb = samp.tile([128, JT, N], BF, tag="mask", name="mask")
                for jt in range(JT):
                    nc.sync.dma_start(out=maskT_sb[:, jt, :], in_=maskT_d[s, jt])
                if s == 0:
                    for kt in range(KT):
                        nc.sync.dma_start(out=wbig2_sb[:, kt, :], in_=wbig2_d[kt])
                        nc.sync.dma_start(out=warep2_sb[:, kt, :], in_=warep2_d[kt])
                    nc.sync.dma_start(out=ident_sb, in_=ident_d[:, :])
                whsb1 = samp.tile([128, JT, H * 65], BF, tag="whsb1", name="whsb1")
                tc1 = samp.tile([128, JT, H], F32, tag="tc1", name="tc1")
                for jt in range(JT):
                    wm_ps = psA.tile([128, H * 65 + H], F32, tag="big", name="wm")
                    for kt in range(KT):
                        nc.tensor.matmul(
                            wm_ps,
                            xT_sb[:, kt, jt * 128 : (jt + 1) * 128],
                            wbig1_sb[:, kt, :],
                            start=(kt == 0),
                            stop=(kt == KT - 1),
                        )
                    nc.scalar.copy(whsb1[:, jt, :], wm_ps[:, 0 : H * 65])
                    nc.vector.memset(whsb1[:, jt, FH : H * 65 : 65], 1.0)
                    nc.scalar.copy(tc1[:, jt, :], wm_ps[:, H * 65 : H * 65 + H])
                rc1 = samp.tile([128, JT, H], F32, tag="rc1", name="rc1")
                fc1 = samp.tile([128, JT, H], F32, tag="fc1", name="fc1")
                nc.scalar.activation(rc1, tc1, AF.Exp, scale=-0.8)
                nc.scalar.activation(fc1, tc1, AF.Exp, scale=1.0)
                hcatT = samp.tile([128, KT, N], BF, tag="hcat", name="hcat")
                d.update(xT_sb=xT_sb, maskT_sb=maskT_sb, whsb1=whsb1,
                         rc1=rc1, fc1=fc1, hcatT=hcatT)
                d["insts"] = {}
                for h in range(H):
                    def emit_l1(o_norm, d=d, h=h):
                        tp_ps = psT.tile([FH, IB, 128], BF, tag="tp", name="tp")
                        for ib in range(IB):
                            nc.tensor.transpose(
                                tp_ps[:, ib, :], o_norm[:, ib, :], ident_sb
                            )
                        dst = d["hcatT"][(h % 2) * 64 : (h % 2) * 64 + 64, h // 2, :]
                        nc.scalar.copy(dst, tp_ps)

                    d["insts"][h] = _Inst(
                        nc, pools, maskT_sb,
                        {
                            "rep": lambda kt, h=h: warep1_sb[:, kt, h * 128 : (h + 1) * 128],
                            "rhs": lambda kt, d=d: d["xT_sb"][:, kt, :],
                            "wh": lambda jt, d=d, h=h: d["whsb1"][:, jt, h * 65 : (h + 1) * 65],
                            "rcol": lambda jt, d=d, h=h: d["rc1"][:, jt, h : h + 1],
                            "fcol": lambda jt, d=d, h=h: d["fc1"][:, jt, h : h + 1],
                        },
                        BF, emit_l1,
                    )
                o2h = {}
                d["o2h"] = o2h

                def emit_l2(o_norm, o2h=o2h):
                    o2h["o2n"] = o_norm

                d["insts"]["L2"] = _Inst(
                    nc, pools, maskT_sb,
                    {
                        "rep": lambda kt: warep2_sb[:, kt, :],
                        "rhs": lambda kt, d=d: d["hcatT"][:, kt, :],
                        "wh": lambda jt, d=d: d["whsb2"][:, jt, :],
                        "rcol": lambda jt, d=d: d["rc2"][:, jt, :],
                        "fcol": lambda jt, d=d: d["fc2"][:, jt, :],
                    },
                    F32, emit_l2,
                )

            def WH2a(s):
                """L2 Wh matmuls; t columns via DVE; r/F exps. The whsb2
                SBUF copies are deferred to WH2b (they gate only P2(L2)),
                keeping the TL(s,3) -> P1b(s,L2) chain short."""
                d = st[s]
                whsb2 = samp.tile([128, JT, 65], BF, tag="whsb2", name="whsb2")
                tc2 = samp.tile([128, JT, 1], F32, tag="tc2", name="tc2")
                hcatT = d["hcatT"]
                d["wm2"] = []
                for jt in range(JT):
                    wm_ps = psA.tile([128, 66], F32, tag="big", name="wm2")
                    d["wm2"].append(wm_ps)
                    for kt in range(KT):
                        nc.tensor.matmul(
                            wm_ps,
                            hcatT[:, kt, jt * 128 : (jt + 1) * 128],
                            wbig2_sb[:, kt, :],
                            start=(kt == 0),
                            stop=(kt == KT - 1),
                        )
                    nc.vector.tensor_copy(tc2[:, jt, :], wm_ps[:, 65:66])
                rc2 = samp.tile([128, JT, 1], F32, tag="rc2", name="rc2")
                fc2 = samp.tile([128, JT, 1], F32, tag="fc2", name="fc2")
                nc.scalar.activation(rc2, tc2, AF.Exp, scale=-0.8)
                nc.scalar.activation(fc2, tc2, AF.Exp, scale=1.0)
                d.update(whsb2=whsb2, rc2=rc2, fc2=fc2)

            def WH2b(s):
                d = st[s]
                whsb2 = d["whsb2"]
                for jt in range(JT):
                    nc.scalar.copy(whsb2[:, jt, 0:FOUT], d["wm2"][jt][:, 0:FOUT])
                    nc.vector.memset(whsb2[:, jt, FOUT : FOUT + 1], 1.0)
                d["wm2"] = None

            def ELU(s):
                """elu(x) = relu(x) + min(exp(x), 1) - 1; -1 folded into the
                post-reduce scale. Mean over nodes (= partitions) on the PE."""
                d = st[s]
                o2n = d["o2h"]["o2n"]
                ex = tailp.tile([128, IB, FH], F32, tag="ex", name="ex")
                nc.scalar.activation(ex, o2n, AF.Exp)
                bmax = tailp.tile([128, IB, FH], F32, tag="bmax", name="bmax")
                nc.scalar.activation(bmax, o2n, AF.Relu)
                eluv = tailp.tile([128, IB, FH], BF, tag="eluv", name="eluv")
                nc.vector.scalar_tensor_tensor(eluv, ex, 1.0, bmax, OP.min, OP.add)
                mean_ps = psA.tile([FH, 1], F32, tag="big", name="mean")
                for ib in range(IB):
                    nc.tensor.matmul(
                        mean_ps,
                        eluv[:, ib, :],
                        ones128_sb,
                        start=(ib == 0),
                        stop=(ib == IB - 1),
                    )
                outc = tailp.tile([FH, 1], F32, tag="outc", name="outc")
                nc.vector.tensor_scalar(outc, mean_ps, 1.0 / N, -1.0, OP.mult, OP.add)
                nc.sync.dma_start(out=out_d[s].rearrange("(f a) -> f a", a=1), in_=outc)

            def P1a(s, k):
                st[s]["insts"][k].phase1a()

            def P1b(s, k):
                st[s]["insts"][k].phase1b()

            def P2(s, k):
                st[s]["insts"][k].phase2()

            def TL(s, k):
                st[s]["insts"][k].tail()

            # ---- emission schedule: sample 0 launches before sample 1's
            # Wh pass (the warmup is DMA-bound); thereafter the two samples
            # interleave at instance granularity. P1a (s-matmul + G) runs
            # ahead of P1b so every G precedes the tails' ACT copies in the
            # in-order queues. At the L2 join, P1a(L2) (which needs only
            # h_cat) precedes the L2 Wh pass, and the whsb2 SBUF copies
            # (WH2b) are deferred off the join-critical path.
            WH1(0)
            P1a(0, 0); P1a(0, 1)
            P1b(0, 0)
            WH1(1)
            P1a(1, 0); P1a(1, 1)
            P1b(1, 0); P1a(0, 2)
            P2(0, 0); P1b(0, 1); P1a(1, 2)
            P2(1, 0); P1b(1, 1); P1a(0, 3)
            TL(0, 0); P2(0, 1); P1b(0, 2); P1a(1, 3)
            TL(1, 0); P2(1, 1); P1b(1, 2)
            TL(0, 1); P2(0, 2); P1b(0, 3)
            TL(1, 1); P2(1, 2); P1b(1, 3)
            TL(0, 2); P2(0, 3)
            TL(1, 2); P2(1, 3)
            TL(0, 3); P1a(0, "L2"); WH2a(0)
            TL(1, 3); P1a(1, "L2"); WH2a(1)
            P1b(0, "L2"); WH2b(0)
            P1b(1, "L2"); WH2b(1)
            P2(0, "L2")
            P2(1, "L2")
            TL(0, "L2"); ELU(0)
            TL(1, "L2"); ELU(1)

    nc.finalize()
    return nc


_NC_CACHE = None


def _prep_host(x, adj, W_heads, a_heads, W_out, a_out):
    xT = np.ascontiguousarray(np.asarray(x, np.float32).transpose(0, 2, 1)).astype(BF16)
    xT = xT.reshape(B, KT, 128, N)
    maskT = (np.asarray(adj) > 0).transpose(0, 2, 1).astype(BF16)  # [B, j, i]
    maskT = np.ascontiguousarray(maskT).reshape(B, JT, 128, N)

    W_heads = np.asarray(W_heads, np.float32)
    a_heads = np.asarray(a_heads, np.float32)
    W_out = np.asarray(W_out, np.float32)
    a_out = np.asarray(a_out, np.float32)

    wbig1 = np.zeros((FIN, H * 65 + H), dtype=np.float32)
    warep1 = np.zeros((FIN, H * 128), dtype=np.float32)
    for h in range(H):
        Wh_ = W_heads[h]
        wbig1[:, h * 65 : h * 65 + FH] = Wh_
        wbig1[:, H * 65 + h] = Wh_ @ a_heads[h, FH:, 0]
        warep1[:, h * 128 : (h + 1) * 128] = (Wh_ @ a_heads[h, :FH, 0])[:, None]
    wbig2 = np.zeros((FIN, 66), dtype=np.float32)
    wbig2[:, 0:FOUT] = W_out
    wbig2[:, 65] = W_out @ a_out[FOUT:, 0]
    warep2 = np.repeat((W_out @ a_out[:FOUT, 0])[:, None], 128, axis=1)

    shared = {
        "wbig1": wbig1.astype(BF16).reshape(KT, 128, H * 65 + H),
        "warep1": warep1.astype(BF16).reshape(KT, 128, H * 128),
        "wbig2": wbig2.astype(BF16).reshape(KT, 128, 66),
        "warep2": warep2.astype(BF16).reshape(KT, 128, 128),
        "ident": np.eye(128, dtype=np.float32).astype(BF16),
    }
    in_maps = []
    for c in range(NCORES):
        sl = slice(c * SPC, (c + 1) * SPC)
        m = {"xT": np.ascontiguousarray(xT[sl]), "maskT": np.ascontiguousarray(maskT[sl])}
        m.update(shared)
        in_maps.append(m)
    return in_maps


def kernel(x, adj, W_heads, a_heads, W_out, a_out, _trace=False):
    global _NC_CACHE
    if _NC_CACHE is None:
        _NC_CACHE = _build_nc()
    nc = _NC_CACHE
    in_maps = _prep_host(x, adj, W_heads, a_heads, W_out, a_out)
    res = run_bass_kernel_spmd(nc, in_maps, core_ids=list(range(NCORES)), trace=_trace)
    out = np.concatenate([res.results[c]["out"] for c in range(NCORES)], axis=0)
    if _trace:
        kernel._last_results = res
    return out.astype(np.float32)


# revision 28
# speedup vs baseline: 1.2200x; 1.0414x over previous
"""GAT (2-layer, 4-head) Bass kernel for Trainium2, data-parallel over 8 NeuronCores.

Math (per sample b, per attention instance with weights W, a = [a1; a2]):
    Wh = h @ W                      [N, F]
    s  = Wh @ a1   (per-dst-node i score part)
    t  = Wh @ a2   (per-src-node j score part)
    e[i,j]   = leaky_relu(s[i] + t[j], 0.2)
    att      = softmax_j(where(adj[i,j] > 0, e, -9e15))
    out[i]   = sum_j att[i,j] * Wh[j]

Key factorization: exp(lrelu(z)) = max(e^z, e^{0.2 z}) for z = s_i + t_j, so
    p[j,i] = m * max(e^{s_i} e^{t_j}, e^{0.2 s_i} e^{0.2 t_j})
           = m * e^{0.2 s_i} * max(e^{0.8 s_i + t_j}, e^{0.2 t_j})
The e^{0.2 s_i} factor is constant along the softmax axis (j) and cancels in
normalization, so the kernel computes only
    p'[j,i] = m[j,i] * max(G[i], r[j]) * F[j]
with G = e^{0.8 s} (one [128,N] ACT exp per instance, via the PE-replicated
s matmul), r = e^{-0.8 t}, F = e^{t} (tiny per-node columns). Per N^2-tile:
one DVE tensor_scalar (max with r-column, mult by F-column) and one native
tensor_tensor mask multiply (2x DVE perf mode). No custom DVE ops, no
per-tile ACT work.

Attention-apply orientation: the contraction over j runs with the p' tile
[j, i-chunk] as the PE stationary and the small [Wh | ones] block moving, so
the output lands as O[i, blk, f] with the softmax row-sum in column 64 --
i.e. BOTH the output and the row-sum are per-i-PARTITION. The reciprocal
then runs on a [128, 8] column (free-size 8, ~100x cheaper than a [1, N]
row) and normalization fuses into the PSUM->SBUF copy as ACT Copy with a
per-partition scale AP. Layer-1 heads are transposed back to h_cat^T
[feat, i] with PE transpose blocks; layer 2 consumes O[i, f] directly
(elu elementwise, mean over nodes via a PE ones-column contraction).

Scheduling: each instance is split into phase1 (s matmul + G exp + score
tensor_scalars + mask multiplies -> p'), phase2 (the 64 attention matmuls),
and tail (reciprocal + normalize + transpose). The emission order software-
pipelines phase1 two instances ahead of phase2 and interleaves the next
sample's L1 with the current sample's L2, so the in-order engine queues
stay full across the layer joins. PSUM is budgeted to exactly 8 banks:
s-halves [128,512] (1 bank x2 bufs), transpose staging (1 x2), attention
outputs split at the bank boundary into two [128,4,65] tiles (1 x2 x2).
"""

import os
import sys

import numpy as np

if not os.path.isdir(os.path.join(os.path.dirname(os.path.abspath(__file__)), "concourse")):
    for _p in ("/opt/trn_rl_repo", os.path.expanduser("~/.axon_site/_ro/trn_rl_repo")):
        if os.path.isdir(_p) and _p not in sys.path:
            sys.path.append(_p)

import ml_dtypes  # noqa: E402

import concourse.bacc as bacc  # noqa: E402
import concourse.tile as tile  # noqa: E402
from concourse import mybir  # noqa: E402
from concourse.bass_utils import run_bass_kernel_spmd  # noqa: E402

BF16 = ml_dtypes.bfloat16

B, N, FIN, FH, H, FOUT = 16, 1024, 256, 64, 4, 64
NCORES = 8
SPC = B // NCORES  # samples per core
KT = FIN // 128    # k tiles (2)
JT = N // 128      # j tiles (8)
IB = N // 128      # i chunks (8)
HB = IB // 2       # i chunks per PSUM tile
ALPHA = 0.2

F32 = mybir.dt.float32
F16 = mybir.dt.float16
BF = mybir.dt.bfloat16
AF = mybir.ActivationFunctionType
OP = mybir.AluOpType
AX = mybir.AxisListType


class _Inst:
    """One attention instance (a head of L1, or L2), emitted in 3 phases."""

    def __init__(self, nc, pools, maskT_sb, spec, out_dt, emit_out):
        self.nc, self.pools, self.maskT_sb = nc, pools, maskT_sb
        self.spec, self.out_dt, self.emit_out = spec, out_dt, emit_out

    def phase1a(self):
        """s matmul halves + G exp halves (PE + ACT front-matter)."""
        nc, spec = self.nc, self.spec
        work, psA = self.pools["work"], self.pools["psA"]

        self.g16 = work.tile([128, N], BF, tag="g16", name="g16")
        for ih in range(2):
            sb_ps = psA.tile([128, 512], F32, tag="big", name="sbh")
            for kt in range(KT):
                nc.tensor.matmul(
                    sb_ps,
                    spec["rep"](kt),
                    spec["rhs"](kt)[:, ih * 512 : (ih + 1) * 512],
                    start=(kt == 0),
                    stop=(kt == KT - 1),
                )
            nc.scalar.activation(
                self.g16[:, ih * 512 : (ih + 1) * 512], sb_ps, AF.Exp, scale=0.8
            )

    def phase1b(self):
        """Score tensor_scalars + mask multiplies -> p' tile (DVE)."""
        nc, spec = self.nc, self.spec
        workbig = self.pools["workbig"]
        pT = workbig.tile([128, JT, N], BF, tag="pt", name="pT")
        self.pT = pT
        g16 = self.g16
        for jt in range(JT):
            nc.vector.tensor_scalar(
                pT[:, jt, :], g16, spec["rcol"](jt), spec["fcol"](jt), OP.max, OP.mult
            )
        for ih in range(2):
            half = slice(ih * (JT // 2), (ih + 1) * (JT // 2))
            nc.vector.tensor_tensor(
                pT[:, half, :], pT[:, half, :], self.maskT_sb[:, half, :], OP.mult
            )

    def phase2(self):
        """O[i, blk, f] (+ rowsum col 64): p' chunks stationary, Wh moving."""
        nc, spec = self.nc, self.spec
        psO = self.pools["psO"]
        self.ot_ps = [
            psO.tile([128, HB, FH + 1], F32, tag=f"ot{half}", name=f"ot{half}")
            for half in range(2)
        ]
        for ib in range(IB):
            for jt in range(JT):
                nc.tensor.matmul(
                    self.ot_ps[ib // HB][:, ib % HB, :],
                    self.pT[:, jt, ib * 128 : (ib + 1) * 128],
                    spec["wh"](jt),
                    start=(jt == 0),
                    stop=(jt == JT - 1),
                )

    def tail(self):
        """Per-partition reciprocal of rowsum cols; normalization rides the
        PSUM->SBUF copies as an ACT per-partition scale. Emitted per PSUM
        half so half 0's tail overlaps half 1's attention matmuls. When
        out_dt is None (L2), the raw (ot_ps, rbc) pair is handed to
        emit_out and normalization fuses into the elu's ACT passes."""
        nc = self.nc
        work = self.pools["work"]
        rsc = work.tile([128, IB], F32, tag="rsc", name="rsc")
        rbc = work.tile([128, IB], F32, tag="rbc", name="rbc")
        for half in range(2):
            hs = slice(half * HB, (half + 1) * HB)
            nc.vector.tensor_copy(rsc[:, hs], self.ot_ps[half][:, :, FH])
            nc.vector.reciprocal_approx_fast(out=rbc[:, hs], in_=rsc[:, hs])
        if self.out_dt is None:
            self.emit_out(self.ot_ps, rbc)
            return
        o_norm = work.tile([128, IB, FH], self.out_dt, tag="onrm", name="onrm")
        for ib in range(IB):
            nc.scalar.activation(
                o_norm[:, ib, :], self.ot_ps[ib // HB][:, ib % HB, 0:FH], AF.Copy,
                scale=rbc[:, ib : ib + 1],
            )
        self.emit_out(o_norm)


def _build_nc():
    nc = bacc.Bacc()

    xT_d = nc.declare_dram_parameter("xT", [SPC, KT, 128, N], BF, isOutput=False)
    maskT_d = nc.declare_dram_parameter("maskT", [SPC, JT, 128, N], BF, isOutput=False)
    wbig1_d = nc.declare_dram_parameter("wbig1", [KT, 128, H * 65 + H], BF, isOutput=False)
    warep1_d = nc.declare_dram_parameter("warep1", [KT, 128, H * 128], BF, isOutput=False)
    wbig2_d = nc.declare_dram_parameter("wbig2", [KT, 128, 66], BF, isOutput=False)
    warep2_d = nc.declare_dram_parameter("warep2", [KT, 128, 128], BF, isOutput=False)
    ident_d = nc.declare_dram_parameter("ident", [128, 128], BF, isOutput=False)
    out_d = nc.declare_dram_parameter("out", [SPC, FOUT], F32, isOutput=True)

    with tile.TileContext(nc) as tc:
        with (
            tc.tile_pool(name="const", bufs=1) as constp,
            tc.tile_pool(name="samp", bufs=2) as samp,
            tc.tile_pool(name="workbig", bufs=4) as workbig,
            tc.tile_pool(name="work", bufs=4) as work,
            tc.tile_pool(name="tail", bufs=1) as tailp,
            tc.tile_pool(name="psA", bufs=2, space="PSUM") as psA,
            tc.tile_pool(name="psT", bufs=2, space="PSUM") as psT,
            tc.tile_pool(name="psO", bufs=2, space="PSUM") as psO,
        ):
            pools = {"work": work, "workbig": workbig, "psA": psA, "psO": psO}

            wbig1_sb = constp.tile([128, KT, H * 65 + H], BF)
            warep1_sb = constp.tile([128, KT, H * 128], BF)
            wbig2_sb = constp.tile([128, KT, 66], BF)
            warep2_sb = constp.tile([128, KT, 128], BF)
            ident_sb = constp.tile([128, 128], BF)
            for kt in range(KT):
                nc.sync.dma_start(out=warep1_sb[:, kt, :], in_=warep1_d[kt])
                nc.sync.dma_start(out=wbig1_sb[:, kt, :], in_=wbig1_d[kt])
            ones128_sb = constp.tile([128, 1], BF)
            nc.vector.memset(ones128_sb, 1.0)

            # Per-sample state built lazily by the unit functions below.
            st = [dict() for _ in range(SPC)]

            def WH1a(s):
                """DMA inputs; t columns via a skinny matmul (the tail H
                columns of wbig1); r/F exps. Gates only the score TS ops, so
                the first instance starts ~4us in instead of waiting for the
                full Wh pass."""
                d = st[s]
                xT_sb = samp.tile([128, KT, N], BF, tag="xt", name="xt")
                for kt in range(KT):
                    nc.sync.dma_start(out=xT_sb[:, kt, :], in_=xT_d[s, kt])
                maskT_sb = samp.tile([128, JT, N], BF, tag="mask", name="mask")
                for jt in range(JT):
                    nc.sync.dma_start(out=maskT_sb[:, jt, :], in_=maskT_d[s, jt])
                if s == 0:
                    for kt in range(KT):
                        nc.sync.dma_start(out=wbig2_sb[:, kt, :], in_=wbig2_d[kt])
                        nc.sync.dma_start(out=warep2_sb[:, kt, :], in_=warep2_d[kt])
                    nc.sync.dma_start(out=ident_sb, in_=ident_d[:, :])
                tc1 = samp.tile([128, JT, H], F32, tag="tc1", name="tc1")
                t_ps = psA.tile([128, JT, H], F32, tag="big", name="tps")
                for jt in range(JT):
                    for kt in range(KT):
                        nc.tensor.matmul(
                            t_ps[:, jt, :],
                            xT_sb[:, kt, jt * 128 : (jt + 1) * 128],
                            wbig1_sb[:, kt, H * 65 : H * 65 + H],
                            start=(kt == 0),
                            stop=(kt == KT - 1),
                        )
                nc.scalar.copy(tc1, t_ps)
                rc1 = samp.tile([128, JT, H], F32, tag="rc1", name="rc1")
                fc1 = samp.tile([128, JT, H], F32, tag="fc1", name="fc1")
                nc.scalar.activation(rc1, tc1, AF.Exp, scale=-0.8)
                nc.scalar.activation(fc1, tc1, AF.Exp, scale=1.0)
                hcatT = samp.tile([128, KT, N], BF, tag="hcat", name="hcat")
                d.update(xT_sb=xT_sb, maskT_sb=maskT_sb,
                         rc1=rc1, fc1=fc1, hcatT=hcatT)
                d["insts"] = {}
                for h in range(H):
                    def emit_l1(o_norm, d=d, h=h):
                        tp_ps = psT.tile([FH, IB, 128], BF, tag="tp", name="tp")
                        for ib in range(IB):
                            nc.tensor.transpose(
                                tp_ps[:, ib, :], o_norm[:, ib, :], ident_sb
                            )
                        dst = d["hcatT"][(h % 2) * 64 : (h % 2) * 64 + 64, h // 2, :]
                        nc.scalar.copy(dst, tp_ps)

                    d["insts"][h] = _Inst(
                        nc, pools, maskT_sb,
                        {
                            "rep": lambda kt, h=h: warep1_sb[:, kt, h * 128 : (h + 1) * 128],
                            "rhs": lambda kt, d=d: d["xT_sb"][:, kt, :],
                            "wh": lambda jt, d=d, h=h: d["whsb1"][:, jt, h * 65 : (h + 1) * 65],
                            "rcol": lambda jt, d=d, h=h: d["rc1"][:, jt, h : h + 1],
                            "fcol": lambda jt, d=d, h=h: d["fc1"][:, jt, h : h + 1],
                        },
                        BF, emit_l1,
                    )
                o2h = {}
                d["o2h"] = o2h

                def emit_l2(ot_ps, rbc, o2h=o2h):
                    o2h["ot"] = ot_ps
                    o2h["rbc"] = rbc

                d["insts"]["L2"] = _Inst(
                    nc, pools, maskT_sb,
                    {
                        "rep": lambda kt: warep2_sb[:, kt, :],
                        "rhs": lambda kt, d=d: d["hcatT"][:, kt, :],
                        "wh": lambda jt, d=d: d["whsb2"][:, jt, :],
                        "rcol": lambda jt, d=d: d["rc2"][:, jt, :],
                        "fcol": lambda jt, d=d: d["fc2"][:, jt, :],
                    },
                    None, emit_l2,
                )

            def WH1b(s):
                """Full L1 Wh pass -> whsb1 (+ ones columns). Gates only
                the attention-apply matmuls (phase2)."""
                d = st[s]
                xT_sb = d["xT_sb"]
                whsb1 = samp.tile([128, JT, H * 65], BF, tag="whsb1", name="whsb1")
                for jt in range(JT):
                    wm_ps = psA.tile([128, H * 65], F32, tag="big", name="wm")
                    for kt in range(KT):
                        nc.tensor.matmul(
                            wm_ps,
                            xT_sb[:, kt, jt * 128 : (jt + 1) * 128],
                            wbig1_sb[:, kt, 0 : H * 65],
                            start=(kt == 0),
                            stop=(kt == KT - 1),
                        )
                    nc.scalar.copy(whsb1[:, jt, :], wm_ps)
                    nc.vector.memset(whsb1[:, jt, FH : H * 65 : 65], 1.0)
                d["whsb1"] = whsb1

            def WH2a(s):
                """L2 Wh matmuls; t columns via DVE; r/F exps. The whsb2
                SBUF copies are deferred to WH2b (they gate only P2(L2)),
                keeping the TL(s,3) -> P1b(s,L2) chain short."""
                d = st[s]
                whsb2 = samp.tile([128, JT, 65], BF, tag="whsb2", name="whsb2")
                tc2 = samp.tile([128, JT, 1], F32, tag="tc2", name="tc2")
                hcatT = d["hcatT"]
                d["wm2"] = []
                for jt in range(JT):
                    wm_ps = psA.tile([128, 66], F32, tag="big", name="wm2")
                    d["wm2"].append(wm_ps)
                    for kt in range(KT):
                        nc.tensor.matmul(
                            wm_ps,
                            hcatT[:, kt, jt * 128 : (jt + 1) * 128],
                            wbig2_sb[:, kt, :],
                            start=(kt == 0),
                            stop=(kt == KT - 1),
                        )
                    nc.vector.tensor_copy(tc2[:, jt, :], wm_ps[:, 65:66])
                rc2 = samp.tile([128, JT, 1], F32, tag="rc2", name="rc2")
                fc2 = samp.tile([128, JT, 1], F32, tag="fc2", name="fc2")
                nc.scalar.activation(rc2, tc2, AF.Exp, scale=-0.8)
                nc.scalar.activation(fc2, tc2, AF.Exp, scale=1.0)
                d.update(whsb2=whsb2, rc2=rc2, fc2=fc2)

            def WH2b(s):
                d = st[s]
                whsb2 = d["whsb2"]
                for jt in range(JT):
                    nc.scalar.copy(whsb2[:, jt, 0:FOUT], d["wm2"][jt][:, 0:FOUT])
                    nc.vector.memset(whsb2[:, jt, FOUT : FOUT + 1], 1.0)
                d["wm2"] = None

            def ELU(s):
                """elu(x) = relu(x) + min(exp(x), 1) - 1; -1 folded into the
                post-reduce scale, the softmax normalization folded into the
                exp/relu scale operand. Mean over nodes (= partitions) on
                the PE."""
                d = st[s]
                ot_ps, rbc = d["o2h"]["ot"], d["o2h"]["rbc"]
                ex = tailp.tile([128, IB, FH], F32, tag="ex", name="ex")
                bmax = tailp.tile([128, IB, FH], F32, tag="bmax", name="bmax")
                eluv = tailp.tile([128, IB, FH], BF, tag="eluv", name="eluv")
                for ib in range(IB):
                    src_ = ot_ps[ib // HB][:, ib % HB, 0:FH]
                    nc.scalar.activation(
                        ex[:, ib, :], src_, AF.Exp, scale=rbc[:, ib : ib + 1]
                    )
                    nc.scalar.activation(
                        bmax[:, ib, :], src_, AF.Relu, scale=rbc[:, ib : ib + 1]
                    )
                nc.vector.scalar_tensor_tensor(eluv, ex, 1.0, bmax, OP.min, OP.add)
                mean_ps = psA.tile([FH, 1], F32, tag="big", name="mean")
                for ib in range(IB):
                    nc.tensor.matmul(
                        mean_ps,
                        eluv[:, ib, :],
                        ones128_sb,
                        start=(ib == 0),
                        stop=(ib == IB - 1),
                    )
                outc = tailp.tile([FH, 1], F32, tag="outc", name="outc")
                nc.vector.tensor_scalar(outc, mean_ps, 1.0 / N, -1.0, OP.mult, OP.add)
                nc.sync.dma_start(out=out_d[s].rearrange("(f a) -> f a", a=1), in_=outc)

            def P1a(s, k):
                st[s]["insts"][k].phase1a()

            def P1b(s, k):
                st[s]["insts"][k].phase1b()

            def P2(s, k):
                st[s]["insts"][k].phase2()

            def TL(s, k):
                st[s]["insts"][k].tail()

            # ---- emission schedule: sample 0 launches before sample 1's
            # Wh pass (the warmup is DMA-bound); thereafter the two samples
            # interleave at instance granularity. P1a (s-matmul + G) runs
            # ahead of P1b so every G precedes the tails' ACT copies in the
            # in-order queues. At the L2 join, P1a(L2) (which needs only
            # h_cat) precedes the L2 Wh pass, and the whsb2 SBUF copies
            # (WH2b) are deferred off the join-critical path.
            WH1a(0)
            P1a(0, 0); P1a(0, 1)
            P1b(0, 0); WH1b(0)
            WH1a(1)
            P1a(1, 0); P1a(1, 1)
            P1b(1, 0); WH1b(1); P1a(0, 2)
            P2(0, 0); P1b(0, 1); P1a(1, 2)
            P2(1, 0); P1b(1, 1); P1a(0, 3)
            TL(0, 0); P2(0, 1); P1b(0, 2); P1a(1, 3)
            TL(1, 0); P2(1, 1); P1b(1, 2)
            TL(0, 1); P2(0, 2); P1b(0, 3)
            TL(1, 1); P2(1, 2); P1b(1, 3)
            TL(0, 2); P2(0, 3)
            TL(1, 2); P2(1, 3)
            TL(0, 3); P1a(0, "L2"); WH2a(0)
            TL(1, 3); P1a(1, "L2"); WH2a(1)
            P1b(0, "L2"); WH2b(0)
            P1b(1, "L2"); WH2b(1)
            P2(0, "L2")
            P2(1, "L2")
            TL(0, "L2"); ELU(0)
            TL(1, "L2"); ELU(1)

    nc.finalize()
    return nc


_NC_CACHE = None


def _prep_host(x, adj, W_heads, a_heads, W_out, a_out):
    xT = np.ascontiguousarray(np.asarray(x, np.float32).transpose(0, 2, 1)).astype(BF16)
    xT = xT.reshape(B, KT, 128, N)
    maskT = (np.asarray(adj) > 0).transpose(0, 2, 1).astype(BF16)  # [B, j, i]
    maskT = np.ascontiguousarray(maskT).reshape(B, JT, 128, N)

    W_heads = np.asarray(W_heads, np.float32)
    a_heads = np.asarray(a_heads, np.float32)
    W_out = np.asarray(W_out, np.float32)
    a_out = np.asarray(a_out, np.float32)

    wbig1 = np.zeros((FIN, H * 65 + H), dtype=np.float32)
    warep1 = np.zeros((FIN, H * 128), dtype=np.float32)
    for h in range(H):
        Wh_ = W_heads[h]
        wbig1[:, h * 65 : h * 65 + FH] = Wh_
        wbig1[:, H * 65 + h] = Wh_ @ a_heads[h, FH:, 0]
        warep1[:, h * 128 : (h + 1) * 128] = (Wh_ @ a_heads[h, :FH, 0])[:, None]
    wbig2 = np.zeros((FIN, 66), dtype=np.float32)
    wbig2[:, 0:FOUT] = W_out
    wbig2[:, 65] = W_out @ a_out[FOUT:, 0]
    warep2 = np.repeat((W_out @ a_out[:FOUT, 0])[:, None], 128, axis=1)

    shared = {
        "wbig1": wbig1.astype(BF16).reshape(KT, 128, H * 65 + H),
        "warep1": warep1.astype(BF16).reshape(KT, 128, H * 128),
        "wbig2": wbig2.astype(BF16).reshape(KT, 128, 66),
        "warep2": warep2.astype(BF16).reshape(KT, 128, 128),
        "ident": np.eye(128, dtype=np.float32).astype(BF16),
    }
    in_maps = []
    for c in range(NCORES):
        sl = slice(c * SPC, (c + 1) * SPC)
        m = {"xT": np.ascontiguousarray(xT[sl]), "maskT": np.ascontiguousarray(maskT[sl])}
        m.update(shared)
        in_maps.append(m)
    return in_maps


def kernel(x, adj, W_heads, a_heads, W_out, a_out, _trace=False):
    global _NC_CACHE
    if _NC_CACHE is None:
        _NC_CACHE = _build_nc()
    nc = _NC_CACHE
    in_maps = _prep_host(x, adj, W_heads, a_heads, W_out, a_out)
    res = run_bass_kernel_spmd(nc, in_maps, core_ids=list(range(NCORES)), trace=_trace)
    out = np.concatenate([res.results[c]["out"] for c in range(NCORES)], axis=0)
    if _trace:
        kernel._last_results = res
    return out.astype(np.float32)
